# revision 19
# baseline (speedup 1.0000x reference)
"""Fused BN(inference)+ReLU -> 1x1 conv (512->256) -> 2x2 avgpool on 8 TRN2 cores.

Full inputs in, full output out. Data-parallel over batch (16 -> 2 per core),
BN params + conv weights replicated.

The whole pipeline is HBM-bandwidth-bound, so everything crossing the wire is
fp16 (harness gate is rel_err < 2e-2; fp16 end-to-end lands ~2e-3):
  x, conv weights, and the output stream as fp16; BN params stay fp32 scalars.

Math folding (host side, tiny):
  s = bn_weight / sqrt(bn_var + eps)            [512]  (s >= 0)
  t = bn_bias - bn_mean * s                     [512]
  y = relu(s * x + t)
  avgpool2x2(W @ y) == (0.25 * W) @ sumpool2x2(y)   (pool before matmul: 4x
                                                     fewer matmul FLOPs)
Per 128-channel k-tile the ReLU runs on one of two engines:
  ACT tiles: y = relu(s*x + t) in one activation op (scale+bias native).
  DVE tiles: s>0 lets us fold s into the conv weight column instead:
             y' = max(x + t/s, 0) is one tensor_scalar(add, max) op, and
             wt[:, c] *= s_c. Splitting tiles across both engines keeps
             either one off the critical path while DMA streams x.
"""

import copy as _copy

import numpy as np

import bass_rust
import concourse.bass as bass
import concourse.mybir as mybir
import concourse.tile as tile_mod
from concourse.bass_utils import run_bass_kernel_spmd

EPS = 1e-5

B, C_IN, C_OUT, H, W = 16, 512, 256, 56, 56
N_CORES = 8
B_PC = B // N_CORES          # batches per core
HW = H * W                   # 3136
HWP = (H // 2) * (W // 2)    # 784 pooled spatial
K_TILES = C_IN // 128        # 4
M_TILES = C_OUT // 128       # 2
N_CHUNK = HWP // 2           # 392 (fits one PSUM bank)

# Which (batch, k) tiles do ReLU on the ACT (scalar) engine vs DVE (vector).
# DVE tensor_scalar is ~3x faster than ACT activation (measured 1.03us vs
# 3.0us per [128,3136] tile), but DVE also carries both pool stages
# (~1.94us/tile) -- offloading ~half the relus to the otherwise-idle ACT
# engine balances the two at ~20us each, just under the ~19us x-stream.
# Edge tiles (first/last) stay on DVE so pipeline fill/drain use the fast
# path. GpSimd is left idle on purpose: running it slows DVE ~30% (measured
# SBUF contention).
# NOTE: must be a per-k split (same for every batch) -- the s-fold into the
# shared conv weights is per input channel.
ACT_K = (1, 2)
ACT_RELU = {(b, k) for b in range(B_PC) for k in ACT_K}

_DT = mybir.dt.float32
_DT16 = mybir.dt.float16


# This walrus build enforces per-instruction sync-wait caps that Tile's
# add_semaphores pass does not respect: CTRL-type instructions (Drain, NoOp)
# take no sem-ge waits at all, EventSemaphore takes at most 2, and every
# other instruction takes at most 1. Post-pass: hoist excess waits onto
# EventSemaphore carrier instructions inserted just before the owning
# instruction on the same engine (same blocking semantics - the carrier
# blocks the engine's sequencer until its waits pass).
_CTRL_OPS = ("InstDrain", "InstNoOp")


def _hoist_excess_waits(nc):
    ev_counter = [0]

    def make_carrier(engine, waits):
        ev_counter[0] += 1
        return mybir.InstEventSemaphore(
            name=f"EVHOIST-{ev_counter[0]}",
            engine=engine,
            ins=[],
            outs=[],
            sync_info=bass_rust.SyncInfo(on_wait=waits, on_update=[]),
        )

    new_module = _copy.replace(nc.m, functions=[])
    for function in nc.m.functions:
        new_function = _copy.replace(function, blocks=[])
        new_function.set_allocations_from_list(function.allocations)
        for block in function.blocks:
            new_insts = []
            for ins in block.instructions:
                si = ins.sync_info
                waits = list(si.on_wait) if si is not None else []
                opname = type(ins).__name__
                if opname in _CTRL_OPS:
                    keep = [w for w in waits if w.wait_mode != "sem-ge-imm"]
                    excess = [w for w in waits if w.wait_mode == "sem-ge-imm"]
                else:
                    limit = 2 if opname == "InstEventSemaphore" else 1
                    keep, excess = waits[:limit], waits[limit:]
                if excess:
                    for i in range(0, len(excess), 2):
                        new_insts.append(make_carrier(ins.engine, excess[i : i + 2]))
                    si.on_wait = keep
                new_insts.append(ins)
            new_function.blocks.append(_copy.replace(block, instructions=new_insts))
        new_module.functions.append(new_function)
    nc.m = new_module


def build_bass():
    nc = bass.Bass()

    # Params come pre-transposed from the host into partition-major layouts so
    # their DMAs are fully contiguous.
    x_d = nc.dram_tensor("x", [B_PC, C_IN, H, W], _DT16, kind="ExternalInput")
    s_d = nc.dram_tensor("s", [128, K_TILES], _DT, kind="ExternalInput")
    t_d = nc.dram_tensor("t", [128, K_TILES], _DT, kind="ExternalInput")
    wt_d = nc.dram_tensor(
        "wt", [128, K_TILES, C_OUT], _DT16, kind="ExternalInput"
    )
    out_d = nc.dram_tensor(
        "out", [B_PC, C_OUT, H // 2, W // 2], _DT16, kind="ExternalOutput"
    )

    with tile_mod.TileContext(nc) as tc:
        with (
            tc.tile_pool(name="const", bufs=1) as cpool,
            tc.tile_pool(name="xs", bufs=8) as xpool,
            tc.tile_pool(name="ys", bufs=5) as ypool,
            tc.tile_pool(name="us", bufs=5) as upool,
            tc.tile_pool(name="ps", bufs=6) as ppool,
            tc.tile_pool(name="os", bufs=4) as opool,
            tc.tile_pool(name="psum", bufs=8, space="PSUM") as pspool,
        ):
            # HWDGE FIFO order: tiny BN params first (so the first relu is
            # gated only by its x data, not queued behind an 800 KB
            # transfer), then the first x half-tiles, then the conv weights,
            # then the rest of the x stream.
            s_sb = cpool.tile([128, K_TILES], _DT)
            nc.sync.dma_start(out=s_sb[:], in_=s_d[:])
            t_sb = cpool.tile([128, K_TILES], _DT)
            nc.sync.dma_start(out=t_sb[:], in_=t_d[:])
            wt_sb = cpool.tile([128, K_TILES, C_OUT], _DT16)
            # Trigger the lazy ACT Relu table load now, off the critical path
            warm = cpool.tile([1, 1], _DT)
            nc.scalar.activation(
                warm[:], s_sb[0:1, 0:1], mybir.ActivationFunctionType.Relu
            )

            def emit_chunk(b, k, row0, nrows, psums, first_k, last_k,
                           defer_mm=None):
                """Process input rows [row0, row0+nrows) of k-slice k:
                DMA -> BN+ReLU -> 2x2 sum-pool -> matmul into psum pieces.

                nrows must be a multiple of 28 so pooled columns align with
                an N_CHUNK boundary (nrows//2 * 28 pooled cols per chunk).
                """
                c = row0 // 14
                hc = nrows * W
                x_t = xpool.tile(
                    [128, hc], _DT16, tag="x", name=f"x_{b}_{k}_{c}"
                )
                nc.sync.dma_start(
                    out=x_t[:],
                    in_=x_d[
                        b,
                        k * 128 : (k + 1) * 128,
                        row0 : row0 + nrows,
                    ].rearrange("ch h w -> ch (h w)"),
                )
                y_t = ypool.tile([128, hc], _DT16, tag="y", name=f"y_{b}_{k}_{c}")
                if (b, k) in ACT_RELU:
                    nc.scalar.activation(
                        y_t[:],
                        x_t[:],
                        mybir.ActivationFunctionType.Relu,
                        bias=t_sb[:, k : k + 1],
                        scale=s_sb[:, k : k + 1],
                    )
                else:
                    # y = max(x + t/s, 0); s folded into wt columns host-side
                    nc.vector.tensor_scalar(
                        y_t[:],
                        x_t[:],
                        t_sb[:, k : k + 1],
                        0.0,
                        op0=mybir.AluOpType.add,
                        op1=mybir.AluOpType.max,
                    )
                # H-pairs first: operands are contiguous 56-elem runs
                # (W-pairs first would be stride-2 reads on the big add)
                u_t = upool.tile(
                    [128, hc // 2], _DT16, tag="u", name=f"u_{b}_{k}_{c}"
                )
                yv = y_t[:].rearrange("p (h two w) -> p h two w", two=2, w=W)
                nc.vector.tensor_add(u_t[:], yv[:, :, 0, :], yv[:, :, 1, :])
                # then W-pairs
                p_t = ppool.tile(
                    [128, hc // 4], _DT16, tag="p", name=f"p_{b}_{k}_{c}"
                )
                uv = u_t[:].rearrange("p (a two) -> p a two", two=2)
                nc.vector.tensor_add(p_t[:], uv[:, :, 0], uv[:, :, 1])
                # map this chunk's pooled columns onto psum n-chunk pieces
                if defer_mm is not None:
                    defer_mm.append((b, k, row0, nrows, p_t, first_k, last_k))
                else:
                    emit_mm(b, k, row0, nrows, p_t, first_k, last_k, psums)

            def emit_mm(b, k, row0, nrows, p_t, first_k, last_k, psums):
                pooled0 = (row0 // 2) * (W // 2)  # global pooled col offset
                pooled_w = (nrows // 2) * (W // 2)
                for m in range(M_TILES):
                    off = 0
                    while off < pooled_w:
                        g = pooled0 + off  # global pooled col
                        n = g // N_CHUNK
                        col = g % N_CHUNK
                        width = min(N_CHUNK - col, pooled_w - off)
                        if first_k and (m, n) not in psums:
                            psums[(m, n)] = pspool.tile(
                                [128, N_CHUNK],
                                _DT,
                                tag="psum",
                                name=f"psum_{b}_{m}_{n}",
                            )
                        nc.tensor.matmul(
                            psums[(m, n)][:, col : col + width],
                            wt_sb[:, k, m * 128 : (m + 1) * 128],
                            p_t[:, off : off + width],
                            start=(first_k and col == 0),
                            stop=(last_k and col + width == N_CHUNK),
                            skip_group_check=True,
                        )
                        off += width

            out_v = out_d[:].rearrange("bb o h w -> bb o (h w)")
            all_psums = {0: {}, 1: {}}
            # First x tile as two half-chunks with their matmuls deferred:
            # the x halves hit the HWDGE FIFO right after the tiny s/t
            # params, the wt transfer slots in behind them, and the deferred
            # matmuls (PE is idle this early anyway) are emitted after the
            # wt DMA so program order matches data order.
            pending = []
            for q in range(2):
                emit_chunk(0, 0, q * 28, 28, all_psums[0], True, False,
                           defer_mm=pending)
            nc.sync.dma_start(out=wt_sb[:], in_=wt_d[:])
            for args in pending:
                emit_mm(*args, all_psums[0])

            def emit_outputs(b, psums, eng):
                for m in range(M_TILES):
                    # PSUM -> SBUF fp32->fp16 (GPSIMD cannot read PSUM, so
                    # these live on ACT); one out-DMA per (b, m)
                    o_t = opool.tile(
                        [128, HWP], _DT16, tag="o", name=f"o_{b}_{m}"
                    )
                    nc.scalar.copy(o_t[:, 0:N_CHUNK], psums[(m, 0)][:])
                    nc.scalar.copy(o_t[:, N_CHUNK:HWP], psums[(m, 1)][:])
                    eng.dma_start(
                        out=out_v[b, m * 128 : (m + 1) * 128, :],
                        in_=o_t[:],
                    )

            for k in range(1, K_TILES):
                emit_chunk(0, k, 0, H, all_psums[0], False, k == K_TILES - 1)
            for k in range(K_TILES - 1):
                emit_chunk(1, k, 0, H, all_psums[1], k == 0, False)
            # b0's psum copies are EMITTED after b1's ACT-relu tiles: the ACT
            # queue is in-order, and copies (gated on b0's last matmul) must
            # not block b1's relus whose data is already in SBUF. b0 outs
            # ride the scalar HWDGE queue (sync is still streaming x);
            # b1 outs go on sync, drained by then.
            emit_outputs(0, all_psums[0], nc.scalar)
            for q in range(2):
                # half chunks at the global pipeline tail: each half covers
                # exactly one psum n-chunk, so the drain chain after the
                # last DMA is half as deep
                emit_chunk(1, K_TILES - 1, q * 28, 28, all_psums[1],
                           False, True)
            emit_outputs(1, all_psums[1], nc.sync)
    _hoist_excess_waits(nc)
    return nc


_NC_CACHE = None


def _get_nc():
    global _NC_CACHE
    if _NC_CACHE is None:
        _NC_CACHE = build_bass()
    return _NC_CACHE


def _prep_host(bn_weight, bn_bias, bn_mean, bn_var, conv_weight):
    s = (bn_weight / np.sqrt(bn_var + EPS)).astype(np.float32)
    t = (bn_bias - bn_mean * s).astype(np.float32)
    wt = (0.25 * conv_weight.T).astype(np.float32)  # [C_IN, C_OUT]
    # DVE tiles: y' = max(x + t/s, 0), s folded into the weight column.
    # Guard: if t/s would overflow fp16's range (pathological tiny s), fall
    # back to the unfolded ACT-compatible params for that channel by leaving
    # it on the ACT list -- but the engine split is per-tile, so instead
    # clamp: channels with tiny s contribute ~nothing x-dependent; clamping
    # t/s keeps the (constant) bias contribution exact via w*s*(t/s) = w*t.
    t_eff = t.copy()
    for k in range(K_TILES):
        if k in ACT_K:
            continue
        cs = slice(k * 128, (k + 1) * 128)
        sk = np.maximum(s[cs], 1e-30)
        tp = t[cs] / sk
        lim = 3.0e4  # fp16 max is 65504; keep |x| + |t/s| clear of it
        big = np.abs(tp) > lim
        if np.any(big):
            # scale s up so t/s fits; folds the same product w*t, only the
            # (negligible) x-dependence of these channels is attenuated
            sk = np.where(big, np.abs(t[cs]) / lim, sk)
            tp = t[cs] / sk
        t_eff[cs] = tp
        wt[cs, :] *= sk[:, None]
    s2 = np.ascontiguousarray(s.reshape(K_TILES, 128).T)
    t2 = np.ascontiguousarray(t_eff.reshape(K_TILES, 128).T)
    wt2 = np.ascontiguousarray(
        wt.astype(np.float16).reshape(K_TILES, 128, C_OUT).transpose(1, 0, 2)
    )
    return s2, t2, wt2


def _install_ntff_hook():
    # The agent image's antenv lacks axon_hooks; synthesize it from the boot
    # shim's ctypes factory so trace=True captures NTFF profiles.
    import sys
    import types

    try:
        import antenv.axon_hooks  # noqa: F401

        return
    except ImportError:
        pass
    from trn_agent_boot.trn_boot import _ntff_profile_via_ctypes

    hook = _ntff_profile_via_ctypes("/opt/axon/libaxon_pjrt.so")
    mod = types.ModuleType("antenv.axon_hooks")
    store = {"h": hook}
    mod.get_axon_ntff_profile_hook = lambda: store["h"]
    mod.set_axon_ntff_profile_hook = lambda h: store.__setitem__("h", h)
    import antenv

    antenv.axon_hooks = mod
    sys.modules["antenv.axon_hooks"] = mod


def kernel(x, bn_weight, bn_bias, bn_mean, bn_var, conv_weight, _trace=False):
    if _trace:
        _install_ntff_hook()
    x16 = np.asarray(x, dtype=np.float16)
    s, t, wt = _prep_host(
        np.asarray(bn_weight, dtype=np.float32),
        np.asarray(bn_bias, dtype=np.float32),
        np.asarray(bn_mean, dtype=np.float32),
        np.asarray(bn_var, dtype=np.float32),
        np.asarray(conv_weight, dtype=np.float32),
    )
    in_maps = [
        {
            "x": np.ascontiguousarray(x16[c * B_PC : (c + 1) * B_PC]),
            "s": s,
            "t": t,
            "wt": wt,
        }
        for c in range(N_CORES)
    ]
    nc = _get_nc()
    res = run_bass_kernel_spmd(
        nc, in_maps, core_ids=list(range(N_CORES)), trace=_trace
    )
    out = np.concatenate(
        [res.results[c]["out"] for c in range(N_CORES)], axis=0
    ).astype(np.float32)
    if _trace:
        return out, res
    return out
